# revision 10
# baseline (speedup 1.0000x reference)
"""Trainium2 Bass kernel for 4D valid convolution (Winograd F(2,3) on c).

x (2,2,32,32,64,64) f32, weight (4,2,3,3,3,3) f32, bias (4,) f32
-> out (2,4,30,30,62,62) f32  (valid cross-correlation + bias)

8 cores = batch(2) x a-quadrant(4). Each core computes
out[b, :, a_sel, :, :, :] from slab x[b, :, a0:a0+10, :, :, :].

Per b-block the conv is a banded matmul: contraction K = (b-window 6,
ci 2, a-window 10) = 120 partitions, M = (co 4, a_out 8, b_out 4) =
128 PSUM partitions; the baseline streamed 9 (kc,kd) tap matmuls of
the full (c,d) plane.  Winograd F(2,3) along c cuts that to 12 streams
of HALF length (4 freqs x 3 kd taps, 31 c-tiles instead of 62 c-cols):

  x~_f[tc, d] = B^T x[2tc..2tc+3, d]          (DVE+GpSimd, bf16)
  m_f  = sum_kd  w~[f,kd]^T @ x~_f[.., kd:kd+62]   (PE, PSUM accum)
  y[2tc]   = m0+m1+m2+bias                     (DVE scalar_tensor_tensor)
  y[2tc+1] = m1-m2-m3+bias                     (GpSimd)

PE columns per core drop 9*62*62*8 -> 12*31*62*8 (2/3 of baseline).
8 garbage warm-up matmuls lift the HAM clock gate to 8/8 before the
first real matmul arrives.  Output goes to DRAM as (parity, tc) planes
per b-block; the host unscrambles (cheap numpy).
"""

import sys

if "/opt/trn_rl_repo" not in sys.path:
    sys.path.insert(0, "/opt/trn_rl_repo")

import ml_dtypes
import numpy as np

BF16 = ml_dtypes.bfloat16

B, CI, CO = 2, 2, 4
A, B2, C, D = 32, 32, 64, 64
AO, BO, CL, DL = 30, 30, 62, 62
K = 3
T = 31  # c-tiles (2 outputs each -> 62)

A0 = [0, 8, 16, 22]
SA = 10  # a-window (8 outputs + 2 halo)
SB = 6  # b-window per block (4 outputs + 2 halo)
NBB = 8  # b_out blocks: 7 full (4 wide) + 1 last (2 wide)
TCH = [(0, 8), (8, 8), (16, 8), (24, 7)]  # (tc0, nct) c-tile chunks

# F(2,3): y = A^T [(G g) . (B^T x)]
G = np.array(
    [[1, 0, 0], [0.5, 0.5, 0.5], [0.5, -0.5, 0.5], [0, 0, 1]], np.float32
)

_CACHE = {}


def _build_weights(weight: np.ndarray, bias: np.ndarray):
    """Banded lhsT per (freq, kd): w~[f,kd] = sum_kc G[f,kc] * tap(kc,kd)."""
    w = weight.astype(np.float32)

    def banded(sa, n_ao, sb, n_bo):
        sa_sel = np.zeros((sa, n_ao, K), np.float32)
        for t in range(K):
            for o in range(n_ao):
                sa_sel[o + t, o, t] = 1.0
        sb_sel = np.zeros((sb, n_bo, K), np.float32)
        for t in range(K):
            for o in range(n_bo):
                sb_sel[o + t, o, t] = 1.0
        P = sb * CI * sa
        M = CO * n_ao * n_bo
        out = np.zeros((P, 12, M), np.float32)
        for f in range(4):
            # wg[co,ci,i,j,kd] = sum_kc G[f,kc] w[co,ci,i,j,kc,kd]
            wg = np.einsum("k,ocijkl->ocijl", G[f], w)
            for kd in range(K):
                m = np.einsum(
                    "dai,ebj,ocij->ecdoab", sa_sel, sb_sel, wg[:, :, :, :, kd]
                )
                out[:, f * 3 + kd, :] = m.reshape(P, M)
        return np.ascontiguousarray(out.reshape(P, 12 * M))

    w_main = banded(SA, 8, SB, 4)  # [120, 12*128]
    w_last = banded(SA, 8, 4, 2)  # [80, 12*64]
    bias_main = np.repeat(bias.astype(np.float32), 32).reshape(128, 1)
    bias_last = np.repeat(bias.astype(np.float32), 16).reshape(64, 1)
    return w_main, w_last, bias_main, bias_last


def _build_program():
    import concourse.bass as bass  # noqa: F401
    import concourse.mybir as mybir
    import concourse.tile as tile
    from concourse import bacc

    f32 = mybir.dt.float32
    bf16 = mybir.dt.bfloat16
    ADD = mybir.AluOpType.add
    SUB = mybir.AluOpType.subtract

    nc = bacc.Bacc("TRN2", target_bir_lowering=False, debug=False, num_devices=8)
    xs = nc.dram_tensor("x_slab", [B2, CI, SA, C, D], bf16, kind="ExternalInput")
    wm = nc.dram_tensor("w_main", [120, 12 * 128], bf16, kind="ExternalInput")
    wl = nc.dram_tensor("w_last", [80, 12 * 64], bf16, kind="ExternalInput")
    bm = nc.dram_tensor("bias_main", [128, 1], f32, kind="ExternalInput")
    bl = nc.dram_tensor("bias_last", [64, 1], f32, kind="ExternalInput")
    # per-block (parity, c-tile) planes; host unscrambles
    out = nc.dram_tensor("out_blocks", [NBB, 128, 2, T * DL], f32, kind="ExternalOutput")

    with tile.TileContext(nc) as tc:
        with (
            tc.tile_pool(name="w", bufs=1) as wpool,
            tc.tile_pool(name="rhs", bufs=2) as rpool,
            tc.tile_pool(name="xt", bufs=2) as xpool,
            tc.tile_pool(name="psum", bufs=2, space="PSUM") as ppool,
            tc.tile_pool(name="y", bufs=3) as ypool,
            tc.tile_pool(name="sc", bufs=3) as spool,
        ):
            # weights ride the GpSimd DGE queue: off the critical rhs path
            w_main_t = wpool.tile([120, 12 * 128], bf16)
            nc.gpsimd.dma_start(w_main_t[:], wm[:])
            w_last_t = wpool.tile([80, 12 * 64], bf16)
            bias_main_t = wpool.tile([128, 1], f32)
            bias_last_t = wpool.tile([64, 1], f32)
            nc.gpsimd.dma_start(bias_main_t[:], bm[:])
            nc.gpsimd.dma_start(w_last_t[:], wl[:])
            nc.gpsimd.dma_start(bias_last_t[:], bl[:])

            # PE warm-up: 8 garbage matmuls (no input deps) so the HAM
            # clock gate is at 8/8 by the time the first real MM arrives
            wu = wpool.tile([128, 512], bf16)
            nc.vector.memset(wu[:], 0)
            ps_w = ppool.tile([128, 4, 512], f32, tag="ps")
            for _ in range(8):
                nc.tensor.matmul(
                    ps_w[:, 0, :], wu[:, :128], wu[:, :], start=True, stop=True
                )

            def fwd(engines, xt, rhs4, tc0, nct):
                # x~0 = x[2t]-x[2t+2]; x~1 = x[2t+1]+x[2t+2]
                # x~2 = x[2t+2]-x[2t+1]; x~3 = x[2t+1]-x[2t+3]
                X0 = rhs4[:, tc0 : tc0 + nct, 0, :]
                X1 = rhs4[:, tc0 : tc0 + nct, 1, :]
                X2 = rhs4[:, tc0 + 1 : tc0 + nct + 1, 0, :]
                X3 = rhs4[:, tc0 + 1 : tc0 + nct + 1, 1, :]
                sl = lambda f: xt[:, f, tc0 : tc0 + nct, :]
                engines[0].tensor_sub(sl(0), X0, X2)
                engines[1].tensor_add(sl(1), X1, X2)
                engines[0].tensor_sub(sl(2), X2, X1)
                engines[1].tensor_sub(sl(3), X1, X3)

            CP = 18  # priority c-cols: chunk 0 uses c in [0, 18)

            rhs_prev = xt_prev = None
            for bb in range(NBB):
                b0 = bb * 4
                wb = SB if bb < NBB - 1 else 4  # b-window width
                wbo = 4 if bb < NBB - 1 else 2  # b_out width
                P = CI * SA * wb  # 120 or 80
                M = CO * 8 * wbo  # 128 or 64

                if bb == 0:
                    rhs_t = rpool.tile([P, C * D], bf16, tag="rhs")
                    # priority: c<CP feeds chunk 0; halves on Sync+ACT queues
                    h = wb // 2  # db halves on the two queues
                    for lo, hi, q in ((0, h, nc.sync), (h, wb, nc.scalar)):
                        q.dma_start(
                            rhs_t[lo * 20 : hi * 20, : CP * D],
                            xs[lo:hi, :, :, :CP].rearrange(
                                "b ci a c d -> (b ci a) (c d)"
                            ),
                        )
                    for lo, hi, q in ((0, h, nc.scalar), (h, wb, nc.sync)):
                        q.dma_start(
                            rhs_t[lo * 20 : hi * 20, CP * D :],
                            xs[lo:hi, :, :, CP:].rearrange(
                                "b ci a c d -> (b ci a) (c d)"
                            ),
                        )
                    xt_t = xpool.tile([P, 4, T, D], bf16, tag="xt")
                else:
                    rhs_t, xt_t = rhs_prev, xt_prev
                rhs4 = rhs_t.rearrange("p (c2 two d) -> p c2 two d", two=2, d=D)

                # prefetch next block's slab (halves on Sync + ACT queues)
                if bb + 1 < NBB:
                    b0n = (bb + 1) * 4
                    wbn = SB if bb + 1 < NBB - 1 else 4
                    Pn = CI * SA * wbn
                    rhs_prev = rpool.tile([Pn, C * D], bf16, tag="rhs")
                    hn = wbn // 2
                    for lo, hi, q in ((0, hn, nc.sync), (hn, wbn, nc.scalar)):
                        q.dma_start(
                            rhs_prev[lo * 20 : hi * 20, :],
                            xs[b0n + lo : b0n + hi].rearrange(
                                "b ci a c d -> (b ci a) (c d)"
                            ),
                        )
                    xt_prev = xpool.tile([Pn, 4, T, D], bf16, tag="xt")
                    rhs4n = rhs_prev.rearrange("p (c2 two d) -> p c2 two d", two=2, d=D)

                w_t = w_main_t if bb < NBB - 1 else w_last_t
                bias_t = bias_main_t if bb < NBB - 1 else bias_last_t

                if bb == 0:
                    # c-ascending chunked transform so chunk 0 only waits
                    # on the priority DMA; later chunks follow the data
                    for tc0, nct in TCH:
                        fwd((nc.vector, nc.gpsimd), xt_t, rhs4, tc0, nct)

                for ci_, (tc0, nct) in enumerate(TCH):
                    N = nct * DL
                    ps = ppool.tile([128, 4, 512], f32, tag="ps")
                    for f in range(4):
                        pv = ps[:M, f, :N].rearrange("m (c d) -> m c d", c=nct)
                        for kd in range(K):
                            rv = xt_t[:, f, tc0 : tc0 + nct, kd : kd + DL]
                            nc.tensor.matmul(
                                pv,
                                w_t[:, (f * 3 + kd) * M : (f * 3 + kd + 1) * M],
                                rv,
                                start=(kd == 0),
                                stop=(kd == 2),
                            )
                    # HW: ops read at most ONE operand from PSUM; GpSimd
                    # cannot read PSUM at all.  ACT evicts t=m1+bias,
                    # s2=m2, s3=m3; DVE: y0=(m0+t)+m2; GPS: y1=(t-s2)-s3
                    y = ypool.tile([128, 2, 496], f32, tag="y")
                    sc = spool.tile([128, 3, 496], f32, tag="sc")
                    t_ = sc[:M, 0, :N]
                    s2_ = sc[:M, 1, :N]
                    s3_ = sc[:M, 2, :N]
                    ID = mybir.ActivationFunctionType.Identity
                    nc.scalar.activation(t_, ps[:M, 1, :N], ID, bias=bias_t[:M])
                    nc.scalar.activation(s2_, ps[:M, 2, :N], ID)
                    nc.scalar.activation(s3_, ps[:M, 3, :N], ID)
                    nc.vector.tensor_add(y[:M, 0, :N], ps[:M, 0, :N], t_)
                    nc.vector.tensor_add(y[:M, 0, :N], ps[:M, 2, :N], y[:M, 0, :N])
                    nc.gpsimd.tensor_sub(y[:M, 1, :N], t_, s2_)
                    nc.gpsimd.tensor_sub(y[:M, 1, :N], y[:M, 1, :N], s3_)
                    # stores alternate Sync/ACT queues (ACT also evicts)
                    q = nc.sync if ci_ % 2 == 0 else nc.scalar
                    q.dma_start(
                        out[bb, :M, :, tc0 * DL : (tc0 + nct) * DL], y[:M, :, :N]
                    )
                    # next block's forward transform after this block's
                    # chunk 1: early enough to beat the PE, late enough
                    # not to block evictions in the engine FIFOs
                    if ci_ == 1 and bb + 1 < NBB:
                        fwd((nc.vector, nc.gpsimd), xt_prev, rhs4n, 0, T)

    nc.compile()
    return nc


def kernel(x: np.ndarray, weight: np.ndarray, bias: np.ndarray) -> np.ndarray:
    from concourse.bass_utils import run_bass_kernel_spmd

    if "nc" not in _CACHE:
        _CACHE["nc"] = _build_program()
    nc = _CACHE["nc"]

    w_main, w_last, bias_main, bias_last = _build_weights(weight, bias)
    x_bf = x.astype(BF16)
    w_main = w_main.astype(BF16)
    w_last = w_last.astype(BF16)

    in_maps = []
    for core in range(8):
        b, q = divmod(core, 4)
        a0 = A0[q]
        in_maps.append(
            {
                "x_slab": np.ascontiguousarray(
                    x_bf[b, :, a0 : a0 + SA].transpose(2, 0, 1, 3, 4)
                ),
                "w_main": w_main,
                "w_last": w_last,
                "bias_main": bias_main,
                "bias_last": bias_last,
            }
        )

    res = run_bass_kernel_spmd(nc, in_maps, core_ids=list(range(8)))
    _CACHE["last_result"] = res

    out = np.empty((B, CO, AO, BO, CL, DL), np.float32)
    for core in range(8):
        b, q = divmod(core, 4)
        slab = _unscramble(res.results[core]["out_blocks"])  # (4, 8, 30, 62, 62)
        if q < 3:
            out[b, :, 8 * q : 8 * q + 8] = slab
        else:
            out[b, :, 24:30] = slab[:, 2:8]
    return out


def _unscramble(blocks: np.ndarray) -> np.ndarray:
    """[NBB, 128, 2, T*62] (parity, c-tile) planes -> (4, 8, 30, 62, 62)."""
    slab = np.empty((CO, 8, BO, CL, DL), np.float32)
    for bb in range(NBB):
        wbo = 4 if bb < NBB - 1 else 2
        m = CO * 8 * wbo
        blk = blocks[bb, :m].reshape(CO, 8, wbo, 2, T, DL)
        # c = 2*tc + parity  ->  [T, 2] c-major
        slab[:, :, bb * 4 : bb * 4 + wbo] = (
            blk.transpose(0, 1, 2, 4, 3, 5).reshape(CO, 8, wbo, CL, DL)
        )
    return slab


# revision 11
# speedup vs baseline: 1.6391x; 1.6391x over previous
"""Trainium2 Bass kernel for 4D valid convolution.

x (2,2,32,32,64,64) f32, weight (4,2,3,3,3,3) f32, bias (4,) f32
-> out (2,4,30,30,62,62) f32  (valid cross-correlation + bias)

Strategy: 8 cores = batch(2) x a-quadrant(4). Each core computes
out[b, :, a_sel, :, :, :] from slab x[b, :, a0:a0+10, :, :, :].

TensorE mapping per core (bf16 inputs, f32 PSUM accumulate):
  K (contraction, partitions) = (b-window=6, ci=2, a-window=10) = 120
  M (psum partitions)         = (co=4, a_out=8, b_out=4) = 128
  N (streamed free dim)       = contiguous (c,d) output pixels, <=496
Host prebuilds banded lhsT matrices (one per (k,l) tap, side by side in
one [120, 9*128] array -> a single DMA); the 9 (k,l) taps accumulate in
PSUM using (c,d)-shifted views of the same SBUF x tile, so each weight
load serves a full 496-column stream and the PE runs back-to-back at
~N cycles/matmul. Loads issue from the Sync DGE queue, stores from the
ACT queue (keeps Sync free to prefetch), evictions (bias add) on DVE.
Output goes to DRAM partition-major per (b-block, c-chunk); the host
unscrambles (SBUF-side multi-dim partition DMAs mislower, so the device
only ever does flat [P, N] stores).

Measured: ~145 us HW exec (8 cores), max rel err ~2.2e-3 vs f32
reference (bf16 input rounding; PE pitch ~210 ns/matmul = bf16
streaming roofline for this shape).
"""

import sys

if "/opt/trn_rl_repo" not in sys.path:
    sys.path.insert(0, "/opt/trn_rl_repo")

import ml_dtypes
import numpy as np

BF16 = ml_dtypes.bfloat16

B, CI, CO = 2, 2, 4
A, B2, C, D = 32, 32, 64, 64
AO, BO, CL, DL = 30, 30, 62, 62
K = 3

# per-core a-slab starts; each core computes 8 output a-rows (q=3 overlaps q=2)
A0 = [0, 8, 16, 22]
SA = 10  # a-window (8 outputs + 2 halo)
SB = 6  # b-window per block (4 outputs + 2 halo)
NBB = 8  # b_out blocks: 7 full (4 wide) + 1 last (2 wide)
NCC = 8  # c chunks: 7 full (8 wide) + 1 last (6 wide)

_CACHE = {}


def _build_weights(weight: np.ndarray, bias: np.ndarray):
    """Banded lhsT matrices per (k,l) tap, plus per-partition bias vectors."""
    w = weight.astype(np.float32)

    def banded(sa, n_ao, sb, n_bo):
        # sel[d, o, t] = 1 if d == o + t
        sa_sel = np.zeros((sa, n_ao, K), np.float32)
        for t in range(K):
            for o in range(n_ao):
                sa_sel[o + t, o, t] = 1.0
        sb_sel = np.zeros((sb, n_bo, K), np.float32)
        for t in range(K):
            for o in range(n_bo):
                sb_sel[o + t, o, t] = 1.0
        # lhsT[(db,ci,da), t=(k,l), (co,ao,bo)] — taps side by side in columns
        # so the whole thing loads with a single 2D DMA into [P, 9*M]
        out = np.zeros((sb * CI * sa, 9, CO * n_ao * n_bo), np.float32)
        for k in range(K):
            for l in range(K):
                wkl = w[:, :, :, :, k, l]  # (co, ci, i, j)
                m = np.einsum("dai,ebj,ocij->ecdoab", sa_sel, sb_sel, wkl)
                out[:, k * 3 + l, :] = m.reshape(sb * CI * sa, CO * n_ao * n_bo)
        return np.ascontiguousarray(out.reshape(sb * CI * sa, 9 * CO * n_ao * n_bo))

    w_main = banded(SA, 8, SB, 4)  # (9, 120, 128)
    w_last = banded(SA, 8, 4, 2)  # (9, 80, 64)
    bias_main = np.repeat(bias.astype(np.float32), 32).reshape(128, 1)
    bias_last = np.repeat(bias.astype(np.float32), 16).reshape(64, 1)
    return w_main, w_last, bias_main, bias_last


def _build_program():
    import concourse.bass as bass  # noqa: F401
    import concourse.mybir as mybir
    import concourse.tile as tile
    from concourse import bacc

    f32 = mybir.dt.float32
    bf16 = mybir.dt.bfloat16

    nc = bacc.Bacc("TRN2", target_bir_lowering=False, debug=False, num_devices=8)
    xs = nc.dram_tensor("x_slab", [B2, CI, SA, C, D], bf16, kind="ExternalInput")
    wm = nc.dram_tensor("w_main", [120, 9 * 128], bf16, kind="ExternalInput")
    wl = nc.dram_tensor("w_last", [80, 9 * 64], bf16, kind="ExternalInput")
    bm = nc.dram_tensor("bias_main", [128, 1], f32, kind="ExternalInput")
    bl = nc.dram_tensor("bias_last", [64, 1], f32, kind="ExternalInput")
    # partition-major blocks: [bb, cc, m, n]; host unscrambles (cheap numpy)
    out = nc.dram_tensor(
        "out_blocks", [NBB, NCC, 128, 8 * DL], f32, kind="ExternalOutput"
    )

    with tile.TileContext(nc) as tc:
        with (
            tc.tile_pool(name="w", bufs=1) as wpool,
            tc.tile_pool(name="rhs", bufs=8) as rpool,
            tc.tile_pool(name="psum", bufs=8, space="PSUM") as ppool,
            tc.tile_pool(name="ot", bufs=4) as opool,
        ):
            # weights ride the GpSimd DGE queue: off the critical rhs path
            w_main_t = wpool.tile([120, 9 * 128], bf16)
            nc.gpsimd.dma_start(w_main_t[:], wm[:])
            # PE warm-up: garbage matmuls (no input deps) lift the HAM
            # clock gate to 8/8 before the first real matmul arrives
            wu = wpool.tile([128, 512], bf16)
            nc.vector.memset(wu[:], 0)
            w_last_t = wpool.tile([80, 9 * 64], bf16)
            bias_main_t = wpool.tile([128, 1], f32)
            bias_last_t = wpool.tile([64, 1], f32)
            nc.gpsimd.dma_start(bias_main_t[:], bm[:])
            nc.gpsimd.dma_start(w_last_t[:], wl[:])
            nc.gpsimd.dma_start(bias_last_t[:], bl[:])

            ps_wu = ppool.tile([128, 496], bf16 and f32, tag="ps")
            for _ in range(8):
                nc.tensor.matmul(
                    ps_wu[:, :496], wu[:, :128], wu[:, :496], start=True, stop=True
                )

            CP = 10  # priority c-cols: chunk 0 reads c in [0, 8+2)

            for bb in range(NBB):
                b0 = bb * 4
                wb = SB if bb < NBB - 1 else 4  # b-window width
                wbo = 4 if bb < NBB - 1 else 2  # b_out width
                P = CI * SA * wb  # 120 or 80
                M = CO * 8 * wbo  # 128 or 64

                rhs_t = rpool.tile([P, C * D], bf16, tag="rhs")
                h = wb // 2
                if bb == 0:
                    # priority: c<CP feeds chunk 0; db halves on Sync+ACT
                    for lo, hi, q in ((0, h, nc.sync), (h, wb, nc.scalar)):
                        q.dma_start(
                            rhs_t[lo * 20 : hi * 20, : CP * D],
                            xs[lo:hi, :, :, :CP].rearrange(
                                "b ci a c d -> (b ci a) (c d)"
                            ),
                        )
                    for lo, hi, q in ((0, h, nc.scalar), (h, wb, nc.sync)):
                        q.dma_start(
                            rhs_t[lo * 20 : hi * 20, CP * D :],
                            xs[lo:hi, :, :, CP:].rearrange(
                                "b ci a c d -> (b ci a) (c d)"
                            ),
                        )
                else:
                    for lo, hi, q in ((0, h, nc.sync), (h, wb, nc.scalar)):
                        q.dma_start(
                            rhs_t[lo * 20 : hi * 20, :],
                            xs[b0 + lo : b0 + hi].rearrange(
                                "b ci a c d -> (b ci a) (c d)"
                            ),
                        )
                rhs3 = rhs_t.rearrange("p (c d) -> p c d", c=C)
                w_t = w_main_t if bb < NBB - 1 else w_last_t
                bias_t = bias_main_t if bb < NBB - 1 else bias_last_t

                for cc in range(NCC):
                    c0 = cc * 8
                    wc = 8 if cc < NCC - 1 else 6
                    N = wc * DL
                    ps = ppool.tile([M, N], f32, tag="ps")
                    for t in range(9):
                        k, l = divmod(t, 3)
                        rv = rhs3[:, c0 + k : c0 + k + wc, l : l + DL]
                        nc.tensor.matmul(
                            ps.rearrange("m (c d) -> m c d", c=wc),
                            w_t[:, t * M : (t + 1) * M],
                            rv,
                            start=(t == 0),
                            stop=(t == 8),
                        )
                    ot = opool.tile([M, N], f32, tag="ot")
                    nc.vector.tensor_scalar_add(ot[:], ps[:], bias_t[:])
                    q = nc.scalar if cc % 2 == 0 else nc.sync
                    q.dma_start(out[bb, cc, :M, :N], ot[:])
    nc.compile()
    return nc


def kernel(x: np.ndarray, weight: np.ndarray, bias: np.ndarray) -> np.ndarray:
    from concourse.bass_utils import run_bass_kernel_spmd

    if "nc" not in _CACHE:
        _CACHE["nc"] = _build_program()
    nc = _CACHE["nc"]

    w_main, w_last, bias_main, bias_last = _build_weights(weight, bias)
    x_bf = x.astype(BF16)
    w_main = w_main.astype(BF16)
    w_last = w_last.astype(BF16)

    in_maps = []
    for core in range(8):
        b, q = divmod(core, 4)
        a0 = A0[q]
        in_maps.append(
            {
                "x_slab": np.ascontiguousarray(
                    x_bf[b, :, a0 : a0 + SA].transpose(2, 0, 1, 3, 4)
                ),
                "w_main": w_main,
                "w_last": w_last,
                "bias_main": bias_main,
                "bias_last": bias_last,
            }
        )

    res = run_bass_kernel_spmd(nc, in_maps, core_ids=list(range(8)))
    _CACHE["last_result"] = res

    out = np.empty((B, CO, AO, BO, CL, DL), np.float32)
    for core in range(8):
        b, q = divmod(core, 4)
        slab = _unscramble(res.results[core]["out_blocks"])  # (4, 8, 30, 62, 62)
        if q < 3:
            out[b, :, 8 * q : 8 * q + 8] = slab
        else:
            out[b, :, 24:30] = slab[:, 2:8]
    return out


def _unscramble(blocks: np.ndarray) -> np.ndarray:
    """[NBB, NCC, 128, 8*62] partition-major blocks -> (4, 8, 30, 62, 62) slab."""
    slab = np.empty((CO, 8, BO, CL, DL), np.float32)
    for bb in range(NBB):
        wbo = 4 if bb < NBB - 1 else 2
        m = CO * 8 * wbo
        for cc in range(NCC):
            wc = 8 if cc < NCC - 1 else 6
            n = wc * DL
            blk = blocks[bb, cc, :m, :n].reshape(CO, 8, wbo, wc, DL)
            slab[:, :, bb * 4 : bb * 4 + wbo, cc * 8 : cc * 8 + wc, :] = blk
    return slab

